# revision 40
# baseline (speedup 1.0000x reference)
"""
Multi-head masked (causal) attention on 8 Trainium2 NeuronCores.

Sharding: core = 2*b + g  (b = batch 0..3, g = head-group 0..1, 6 heads each).
Each core computes, for its batch b and heads [6g, 6g+6):
    q,k,v projections -> causal attention -> partial out-projection
    (rows [384g, 384g+384) of Wo), output written TRANSPOSED [768, S].
Host gathers: out[b] = (part[2b] + part[2b+1]).T + bo.

All matmuls in bf16 (PE 1 cycle/row vs fp32's 4), fp32 PSUM accumulation.
Scores are computed transposed (S^T[sk, sq] = K^T(stationary-ish) x Q^T) so:
  - exp runs on ACT straight out of PSUM (scale=1/8 fused),
  - AV uses V as the stationary operand with an appended ones-column,
    yielding ctx^T[j, sq] AND the softmax denominator in one accumulation,
  - ctx^T is exactly the lhsT layout the out-projection needs.
Causal structure is exploited block-exactly: for key-block ik only
sq >= 128*ik is computed; diagonal 128x128 blocks get a triangular mask
multiply (on GPSIMD) after exp.
"""

import numpy as np
import ml_dtypes

import concourse.bass as bass
import concourse.mybir as mybir
import concourse.tile as tile
from concourse import bacc

BF16 = mybir.dt.bfloat16
F32 = mybir.dt.float32

# Problem constants (hardcoded per contract)
B, S, D = 4, 2048, 768
N_HEADS_TOTAL = 12
HD = 64                      # head dim
H = 6                        # local heads per core
NPAIR = H // 2               # head pairs (Q/K computed 2 heads at a time)
NC_D = D // 128              # contraction chunks over D (6)
NSK = S // 128               # key blocks (16)
BT = 1024                    # query-tile width for the attention phase
NT2 = S // BT                # query tiles (2)
VW = H * (HD + 64)           # v storage: per head [v(64) | ones(64)] (768)
SCALE = 1.0 / np.sqrt(HD)


def _chunks(total, step=512):
    out = []
    n0 = 0
    while n0 < total:
        w = min(step, total - n0)
        out.append((n0, w))
        n0 += w
    return out


def build_nc():
    nc = bacc.Bacc(None, target_bir_lowering=False)

    xT_d = nc.declare_dram_parameter("xT", [D, S], BF16, isOutput=False)
    wq_d = nc.declare_dram_parameter("wq", [D, H * HD], BF16, isOutput=False)
    wk_d = nc.declare_dram_parameter("wk", [D, H * HD], BF16, isOutput=False)
    wv_d = nc.declare_dram_parameter("wv", [D, H * HD], BF16, isOutput=False)
    bq_d = nc.declare_dram_parameter("bq", [128, NPAIR], F32, isOutput=False)
    bk_d = nc.declare_dram_parameter("bk", [128, NPAIR], F32, isOutput=False)
    bv_d = nc.declare_dram_parameter("bv", [1, VW], F32, isOutput=False)
    wo_d = nc.declare_dram_parameter("wo", [H * HD, D], BF16, isOutput=False)
    mask_d = nc.declare_dram_parameter("mask", [128, 128], BF16, isOutput=False)
    outT_d = nc.declare_dram_parameter("outT", [D, S], F32, isOutput=True)

    with tile.TileContext(nc) as tc:
        with (
            tc.tile_pool(name="const", bufs=1) as constp,
            tc.tile_pool(name="big", bufs=1) as bigp,
            tc.tile_pool(name="epool", bufs=4) as epool,
            tc.tile_pool(name="rpool", bufs=2) as rpool,
            tc.tile_pool(name="opool", bufs=3) as opool,
            tc.tile_pool(name="work", bufs=2, space="PSUM") as work,
            tc.tile_pool(name="ctxp", bufs=2, space="PSUM") as ctxp,
        ):
            # ---- constants / weights ----
            mask_sb = constp.tile([128, 128], BF16)
            nc.sync.dma_start(mask_sb[:], mask_d[:])
            bq_sb = constp.tile([128, NPAIR], F32)
            nc.sync.dma_start(bq_sb[:], bq_d[:])
            bk_sb = constp.tile([128, NPAIR], F32)
            nc.sync.dma_start(bk_sb[:], bk_d[:])
            bvb_sb = constp.tile([128, VW], F32)
            nc.sync.dma_start(
                bvb_sb[:, None, :],
                bv_d[:].partition_broadcast(128),
            )

            wq_sb = constp.tile([128, NC_D, H * HD], BF16)
            nc.sync.dma_start(wq_sb[:], wq_d.rearrange("(c p) n -> p c n", p=128))
            wk_sb = constp.tile([128, NC_D, H * HD], BF16)
            nc.sync.dma_start(wk_sb[:], wk_d.rearrange("(c p) n -> p c n", p=128))
            wv_sb = constp.tile([128, NC_D, H * HD], BF16)
            nc.sync.dma_start(wv_sb[:], wv_d.rearrange("(c p) n -> p c n", p=128))
            wo_sb = constp.tile([128, NPAIR, D], BF16)
            nc.sync.dma_start(wo_sb[:], wo_d.rearrange("(c p) n -> p c n", p=128))

            # x^T resident in SBUF: [128, NC_D, S] (split DMAs -> more queues)
            xT_sb = bigp.tile([128, NC_D, S], BF16)
            for c in range(NC_D):
                for hh in range(2):
                    nc.sync.dma_start(
                        xT_sb[:, c, hh * (S // 2):(hh + 1) * (S // 2)],
                        xT_d[c * 128:(c + 1) * 128,
                             hh * (S // 2):(hh + 1) * (S // 2)])

            qT_sb = bigp.tile([128, NPAIR, S], BF16)
            kT_sb = bigp.tile([128, NPAIR, S], BF16)
            v_sb = bigp.tile([128, NSK, VW], BF16)
            ctxT_sb = bigp.tile([128, NPAIR, S], BF16)
            # ones-blocks of v (cols [64,128) per head) — set once on DVE
            nc.vector.memset(
                v_sb[:].rearrange("p s (h c) -> p s h c", h=H)[:, :, :, HD:128],
                1.0,
            )

            def qk_tile(p, which, t):
                dst_sb, w_sb, b_sb = ((qT_sb, wq_sb, bq_sb),
                                      (kT_sb, wk_sb, bk_sb))[which]
                ps = work.tile([128, 1024], F32, tag="work")
                for c in range(NC_D):
                    nc.tensor.matmul(
                        ps[:, 0:512],
                        w_sb[:, c, p * 128:(p + 1) * 128],
                        xT_sb[:, c, t * 512:(t + 1) * 512],
                        start=(c == 0), stop=(c == NC_D - 1),
                    )
                nc.vector.tensor_add(
                    out=dst_sb[:, p, t * 512:(t + 1) * 512],
                    in0=ps[:, 0:512],
                    in1=b_sb[:, p:p + 1].broadcast_to((128, 512)),
                )

            def qk_proj(p):
                for which in range(2):
                    for t in range(S // 512):
                        qk_tile(p, which, t)

            def v_proj(s):
                ps = work.tile([128, 1024], F32, tag="work")
                for c in range(NC_D):
                    nc.tensor.matmul(
                        ps[:, 0:H * HD],
                        xT_sb[:, c, s * 128:(s + 1) * 128],
                        wv_sb[:, c, :],
                        start=(c == 0), stop=(c == NC_D - 1),
                    )
                nc.vector.tensor_add(
                    out=v_sb[:, s, :].rearrange("p (h c) -> p h c", h=H)[:, :, 0:HD],
                    in0=ps[:, 0:H * HD].rearrange("p (h c) -> p h c", h=H),
                    in1=bvb_sb.rearrange("p (h c) -> p h c", h=H)[:, :, 0:HD],
                )

            def attention(p, t2, fillers=None, filler_stride=1):
                # both heads of pair p, interleaved: the two K=64 score
                # matmuls target PE row-groups 0/64 (auto tile_position from
                # lhsT base partition) and run concurrently in the array.
                q0 = t2 * BT
                nik = (q0 + BT) // 128
                ctxs = [ctxp.tile([128, BT], F32, tag="ctx", name=f"ctx{p}{t2}{hf}")
                        for hf in range(2)]
                for ik in range(nik):
                    # slot in one unit of independent PE work to fill the
                    # exp-wait gap (ACT is the binding engine here)
                    if fillers and ik % filler_stride == 0:
                        fillers.pop(0)()
                    sq0 = max(q0, 128 * ik)
                    W = q0 + BT - sq0
                    es = []
                    for half in range(2):
                        hp = slice(half * 64, half * 64 + 64)
                        s_ps = work.tile([128, 1024], F32, tag="work",
                                         name=f"sps{half}")
                        for (n0, w) in _chunks(W):
                            nc.tensor.matmul(
                                s_ps[:, n0:n0 + w],
                                kT_sb[hp, p, ik * 128:(ik + 1) * 128],
                                qT_sb[hp, p, sq0 + n0:sq0 + n0 + w],
                                start=True, stop=True,
                            )
                        e_sb = epool.tile([128, 1024], BF16, tag="e",
                                          name=f"e{half}")
                        nc.scalar.activation(
                            e_sb[:, 0:W], s_ps[:, 0:W],
                            mybir.ActivationFunctionType.Exp, scale=float(SCALE),
                        )
                        if 128 * ik >= q0:
                            nc.gpsimd.tensor_mul(
                                e_sb[:, 0:128], e_sb[:, 0:128], mask_sb[:],
                            )
                        es.append(e_sb)
                    for half in range(2):
                        h = 2 * p + half
                        off = sq0 - q0
                        c0 = off
                        while c0 < BT:
                            w = min(512 - (c0 % 512), BT - c0)
                            nc.tensor.matmul(
                                ctxs[half][:, c0:c0 + w],
                                v_sb[:, ik, h * 128:(h + 1) * 128],
                                es[half][:, c0 - off:c0 - off + w],
                                start=(ik == 0), stop=(ik == nik - 1),
                                skip_group_check=True,
                            )
                            c0 += w
                # normalize: ctx^T /= denom (rows 64:128 hold the denom)
                for half in range(2):
                    hp = slice(half * 64, half * 64 + 64)
                    rsum = rpool.tile([64, BT], F32, tag="rsum")
                    nc.vector.tensor_copy(rsum[:], ctxs[half][64:128, :])
                    rcp = rpool.tile([64, BT], F32, tag="rcp")
                    nc.vector.reciprocal_approx_fast(rcp[:], rsum[:])
                    nc.vector.tensor_mul(
                        ctxT_sb[hp, p, q0:q0 + BT],
                        ctxs[half][0:HD, :],
                        rcp[:],
                    )

            def out_tile(t2, n, n0, w):
                q0 = t2 * BT
                po = work.tile([128, 1024], F32, tag="work")
                for c in range(NPAIR):
                    nc.tensor.matmul(
                        po[:, 0:w],
                        wo_sb[:, c, n * 128:(n + 1) * 128],
                        ctxT_sb[:, c, q0 + n0:q0 + n0 + w],
                        start=(c == 0), stop=(c == NPAIR - 1),
                    )
                ot = opool.tile([128, 512], F32, tag="ot")
                nc.vector.tensor_copy(ot[:, 0:w], po[:, 0:w])
                nc.sync.dma_start(
                    outT_d[n * 128:(n + 1) * 128, q0 + n0:q0 + n0 + w],
                    ot[:, 0:w],
                )

            # Emission order: minimal prologue (pair-0 Q/K + first half of V)
            # so ACT's exp stream starts early; remaining projection work is
            # interleaved between attention steps as PE gap-filler, since the
            # attention phase is ACT-bound and PE would otherwise micro-idle
            # (HAM re-throttle). out_proj(t2=0) fills attention(t2=1).
            import functools
            qk_proj(0)
            for s in range(NSK // 2):
                v_proj(s)
            fillers = []
            for p in (1, 2):
                for which in range(2):
                    for t in range(S // 512):
                        fillers.append(functools.partial(qk_tile, p, which, t))
            fillers.extend(functools.partial(v_proj, s)
                           for s in range(NSK // 2, NSK))
            # t2=0 attention: 3 pairs x 8 iks = 24 slots for 24 fillers
            for p in range(NPAIR):
                attention(p, 0, fillers)
            assert not fillers, f"{len(fillers)} fillers left"
            fillers = [functools.partial(out_tile, 0, n, n0, w)
                       for n in range(D // 128) for (n0, w) in _chunks(BT)]
            for p in range(NPAIR):
                attention(p, 1, fillers, filler_stride=4)
            assert not fillers
            for n in range(D // 128):
                for (n0, w) in _chunks(BT):
                    out_tile(1, n, n0, w)
    nc.finalize()
    return nc


_NC_CACHE = None


def _get_nc():
    global _NC_CACHE
    if _NC_CACHE is None:
        _NC_CACHE = build_nc()
    return _NC_CACHE


def make_in_maps(x, Wq, Wk, Wv, bq, bk, bv, Wo, bo):
    bf16 = ml_dtypes.bfloat16
    tri = np.triu(np.ones((128, 128), np.float32)).astype(bf16)
    in_maps = []
    for core in range(8):
        b, g = core // 2, core % 2
        hs = slice(6 * g, 6 * g + 6)
        xT = np.ascontiguousarray(np.asarray(x[b]).T).astype(bf16)
        wq = np.ascontiguousarray(
            np.asarray(Wq[hs]).transpose(1, 0, 2).reshape(D, H * HD)).astype(bf16)
        wk = np.ascontiguousarray(
            np.asarray(Wk[hs]).transpose(1, 0, 2).reshape(D, H * HD)).astype(bf16)
        wv = np.ascontiguousarray(
            np.asarray(Wv[hs]).transpose(1, 0, 2).reshape(D, H * HD)).astype(bf16)
        bqc = np.zeros((128, NPAIR), np.float32)
        bkc = np.zeros((128, NPAIR), np.float32)
        for p in range(NPAIR):
            bqc[0:64, p] = bq[6 * g + 2 * p]
            bqc[64:128, p] = bq[6 * g + 2 * p + 1]
            bkc[0:64, p] = bk[6 * g + 2 * p]
            bkc[64:128, p] = bk[6 * g + 2 * p + 1]
        bvr = np.zeros((1, VW), np.float32)
        for h in range(H):
            bvr[0, h * 128:h * 128 + HD] = bv[6 * g + h]
        wo = np.ascontiguousarray(Wo[384 * g:384 * (g + 1), :]).astype(bf16)
        in_maps.append({
            "xT": xT, "wq": wq, "wk": wk, "wv": wv,
            "bq": bqc, "bk": bkc, "bv": bvr, "wo": wo, "mask": tri,
        })
    return in_maps


def kernel(x, Wq, Wk, Wv, bq, bk, bv, Wo, bo):
    from concourse.bass_utils import run_bass_kernel_spmd

    nc = _get_nc()
    in_maps = make_in_maps(x, Wq, Wk, Wv, bq, bk, bv, Wo, bo)
    res = run_bass_kernel_spmd(nc, in_maps, list(range(8)))
    out = np.empty((B, S, D), np.float32)
    bo32 = np.asarray(bo, np.float32)
    for b in range(B):
        pT = res.results[2 * b]["outT"] + res.results[2 * b + 1]["outT"]
        out[b] = pT.T + bo32[None, :]
    return out
